# revision 13
# baseline (speedup 1.0000x reference)
"""BinarizedDense TRN2 kernel: out = inputs @ (kernel > 0.5).

inputs [8192, 4096] f32, kernel [4096, 4096] f32 -> out [8192, 4096] f32.

Strategy
--------
Data parallel over tokens: each of the 8 NeuronCores computes a
[1024, 4096] output shard against the full binarized weight matrix.

Single fp16 pass on the PE array. The binarized weights {0,1} are exact
in fp16; the only approximation is rounding the activations to fp16
(~2^-11 relative), which lands the output at ~3e-4 relative error --
far inside the 2e-2 gate -- at 1 column/cycle. (TRN2/cayman has no
uint8 matmul and DoubleRow is fp8-only at 2x, so any scheme with a
correction pass costs the same 1.0 fp16-equivalents; a lone fp8 pass
would be 2x faster but ~3.6e-2 error. 1.0 is the accuracy/ISA floor.)

Layout per core: activations staged transposed ([in_f, tok], K on
partitions), SBUF-resident (32 tiles of [128, 1024] fp16 = 8 MB);
weights stream from HBM once per 512-wide output block from a
host-pretiled contiguous layout [n_blk, k_tile, 128, 512]; PSUM
accumulates over K with the 8 token-tiles of a block in the 8 PSUM
banks; DVE evicts PSUM->SBUF and DMA writes the fp32 output.
Activation loads are emitted just in time inside output block 0 so the
first weight tile isn't queued behind them.
"""
from contextlib import ExitStack

import numpy as np

import concourse.bass as bass
import concourse.tile as tile
from concourse import bacc, mybir
from concourse.bass_utils import run_bass_kernel_spmd

TOKENS, IN_F, OUT_F = 8192, 4096, 4096
N_CORES = 8
TOK = TOKENS // N_CORES      # 1024 tokens per core
P = 128                      # partitions
NT = 512                     # output free-dim tile (one PSUM bank of fp32)
KT = IN_F // P               # 32 contraction tiles
MT = TOK // P                # 8 token tiles per core
NTI = OUT_F // NT            # 8 output blocks

_F16 = mybir.dt.float16
_F32 = mybir.dt.float32

_cached = None


def _ensure_axon_hooks():
    """bass_utils' trace path (trace=True or BASS_TRACE=1) imports
    antenv.axon_hooks, which this image's antenv package lacks. Provide
    it, registering the ctypes NTFF hook when available so profiling
    works; with no hook registered bass_utils degrades gracefully."""
    import sys
    import types
    try:
        import antenv
        if hasattr(antenv, "axon_hooks"):
            return
        mod = types.ModuleType("antenv.axon_hooks")
        _hook = [None]
        mod.set_axon_ntff_profile_hook = lambda h: _hook.__setitem__(0, h)
        mod.get_axon_ntff_profile_hook = lambda: _hook[0]
        sys.modules["antenv.axon_hooks"] = mod
        antenv.axon_hooks = mod
        try:
            from trn_agent_boot.trn_boot import _ntff_profile_via_ctypes
            mod.set_axon_ntff_profile_hook(
                _ntff_profile_via_ctypes("/opt/axon/libaxon_pjrt.so"))
        except Exception:
            pass
    except Exception:
        pass


_ensure_axon_hooks()


def _build():
    nc = bacc.Bacc("TRN2", target_bir_lowering=False, debug=False)
    xhi = nc.dram_tensor("xhi", [IN_F, TOK], _F16, kind="ExternalInput").ap()
    # host-pretiled contiguous weight blocks: [NTI, KT, P, NT]
    whi = nc.dram_tensor("whi", [NTI, KT, P, NT], _F16,
                         kind="ExternalInput").ap()
    out = nc.dram_tensor("out", [TOK, OUT_F], _F32, kind="ExternalOutput").ap()

    with tile.TileContext(nc) as tc:
        with ExitStack() as ctx:
            xp = ctx.enter_context(tc.tile_pool(name="x", bufs=1))
            wp = ctx.enter_context(tc.tile_pool(name="w", bufs=1))
            op = ctx.enter_context(tc.tile_pool(name="o", bufs=8))
            pp = ctx.enter_context(tc.tile_pool(name="p", bufs=8, space="PSUM"))

            his = []   # KT fp16 tiles [P, TOK]
            whs = {}   # weight tiles of the current block



            def load_tiles(n):
                for k in range(KT):
                    if n == 0 and k == 0:
                        # Shrink the first matmul's critical path: split
                        # the k=0 activation tile into the m=0 slice +
                        # the rest, and the first weight tile into two
                        # half-width tiles, so the first matmul waits on
                        # ~96 KB instead of 288 KB.
                        # Issue the first matmul's operands on the idle
                        # Activation engine's DMA queue so they don't sit
                        # behind the Sync engine's descriptor stream.
                        wh0a = wp.tile([P, NT // 2], _F16, tag="wh0a",
                                       name="wh0a", bufs=1)
                        nc.scalar.dma_start(wh0a[:], whi[0, 0, :, :NT // 2])
                        xh0a = xp.tile([P, P], _F16, tag="xh0a",
                                       name="xh0a", bufs=1)
                        nc.scalar.dma_start(xh0a[:], xhi[0:P, 0:P])
                        wh0b = wp.tile([P, NT // 2], _F16, tag="wh0b",
                                       name="wh0b", bufs=1)
                        nc.scalar.dma_start(wh0b[:], whi[0, 0, :, NT // 2:])
                        xh0b = xp.tile([P, TOK - P], _F16, tag="xh0b",
                                       name="xh0b", bufs=1)
                        nc.sync.dma_start(xh0b[:], xhi[0:P, P:TOK])
                        his.append((xh0a, xh0b))
                        wh = (wh0a, wh0b)
                    else:
                        wh = wp.tile([P, NT], _F16, tag="wh",
                                     name=f"wh{n}_{k}", bufs=WB)
                        nc.sync.dma_start(wh[:], whi[n, k])
                        if n == 0:
                            th = xp.tile([P, TOK], _F16, tag="xh",
                                         name=f"xh{k}", bufs=KT)
                            nc.sync.dma_start(
                                th[:], xhi[k * P:(k + 1) * P, :])
                            his.append(th)
                    yield k, wh

            def sta_ap(k, m):
                if k == 0:
                    xh0a, xh0b = his[0]
                    return (xh0a[:] if m == 0
                            else xh0b[:, (m - 1) * P:m * P])
                return his[k][:, m * P:(m + 1) * P]

            def mm(pt, k, m, wh, start, stop):
                sta = sta_ap(k, m)
                if isinstance(wh, tuple):
                    # start only on the first half: the group's first
                    # matmul zeroes the whole 2 KB PSUM region, so a
                    # second start=True would clear half-A's has-written
                    # state and k=1 would overwrite instead of accumulate.
                    nc.tensor.matmul(pt[:, :NT // 2], sta, wh[0][:],
                                     start=start, stop=stop)
                    nc.tensor.matmul(pt[:, NT // 2:], sta, wh[1][:],
                                     start=False, stop=stop)
                else:
                    nc.tensor.matmul(pt[:], sta, wh[:],
                                     start=start, stop=stop)

            def evict(n, m, pt):
                ot = op.tile([P, NT], _F32, tag="o", name=f"o{n}_{m}")
                nc.vector.tensor_copy(ot[:], pt[:])
                nc.sync.dma_start(
                    out[m * P:(m + 1) * P, n * NT:(n + 1) * NT], ot[:])

            WB = KT   # full-block weight ring: deep prefetch, one slot/tile
            KH = KT // 2
            for n in range(NTI):
                pts = [pp.tile([P, NT], _F32, tag="p", name=f"p{n}_{m}")
                       for m in range(MT)]
                last = n == NTI - 1
                if last:
                    # Last block: run the second half of K in m-outer
                    # order so PSUM banks retire one at a time and
                    # evictions + output DMA overlap the remaining
                    # matmuls instead of forming a serial tail.
                    tiles = dict(load_tiles(n))
                    for k in range(KH):
                        for m in range(MT):
                            mm(pts[m], k, m, tiles[k],
                               start=(k == 0), stop=False)
                    for m in range(MT):
                        for k in range(KH, KT):
                            mm(pts[m], k, m, tiles[k],
                               start=False, stop=(k == KT - 1))
                        evict(n, m, pts[m])
                else:
                    for k, wh in load_tiles(n):
                        for m in range(MT):
                            mm(pts[m], k, m, wh,
                               start=(k == 0), stop=(k == KT - 1))
                    for m in range(MT):
                        evict(n, m, pts[m])
    nc.compile()
    return nc


def _get_module():
    global _cached
    if _cached is None:
        _cached = _build()
    return _cached


def _prep_host(inputs: np.ndarray, kernel_w: np.ndarray):
    inputs = np.asarray(inputs, dtype=np.float32)
    kernel_w = np.asarray(kernel_w, dtype=np.float32)

    whi = (kernel_w > 0.5).astype(np.float16)
    # pretile to [NTI, KT, P, NT] so each weight-tile DMA is one
    # contiguous 128 KB read
    whi = np.ascontiguousarray(
        whi.reshape(KT, P, NTI, NT).transpose(2, 0, 1, 3))
    hi = inputs.astype(np.float16)
    return hi, whi


def _run(inputs: np.ndarray, kernel_w: np.ndarray, trace: bool = False):
    nc = _get_module()
    hi, whi = _prep_host(inputs, kernel_w)

    in_maps = []
    for i in range(N_CORES):
        sl = slice(i * TOK, (i + 1) * TOK)
        in_maps.append({
            "xhi": np.ascontiguousarray(hi[sl].T),
            "whi": whi,
        })

    res = None
    last_exc = None
    for attempt in range(3):
        try:
            res = run_bass_kernel_spmd(
                nc, in_maps, core_ids=list(range(N_CORES)), trace=trace)
            break
        except Exception as e:  # transient device wedges have been observed
            last_exc = e
            try:
                import jax
                jax.clear_caches()
                jax.clear_backends()
            except Exception:
                pass
    if res is None:
        raise last_exc
    full = np.concatenate([r["out"] for r in res.results], axis=0)
    return full, res


def kernel(inputs: np.ndarray, kernel: np.ndarray) -> np.ndarray:
    return _run(inputs, kernel)[0]
